# revision 23
# baseline (speedup 1.0000x reference)
# BiLSTM-CRF NLL kernel for 8x Trainium2 NeuronCores (Bass/Tile).
#
# Strategy: data-parallel over batch (16 seqs/core). Per core:
#   P0  embedding gather (indirect DMA) + PE-transpose -> eT [feat, time*batch] bf16
#   P1  BiLSTM layer 0: fused fwd+bwd step pipeline; strip-layout gates in one
#       PSUM bank via 4 tensor-engine column groups; single-func activation
#       (sigmoid(x) = (tanh(x/2)+1)/2, tanh-gate columns pre-doubled host-side);
#       cell state kept as C2=2c, hidden stored as Hh=2h (downstream weights
#       pre-halved host-side); per-step PE transpose of Hh -> hT storage.
#   P2  BiLSTM layer 1 (input = layer-0 output)
#   P3+P4  FC -> emissions em3 = em - 3 (drift fold); fused per-chunk:
#       expem = exp(em3) (bf16, kept), one-hot M1 from tags, gold-path score
#       pieces accumulated via strided reduces + matmuls.
#   P5  CRF partition function in pure exp space:
#         S_t = (exp(trans)^T @ S_{t-1}) * expem_t   (no ACT in the scan loop)
#       logZ_dev = ln(sum_j S_L * exp(end));  v_b = logZ_dev - score_dev
#       (the -3/step drift cancels exactly between logZ_dev and score_dev)
# Host: output = mean over all 128 v_b.
import sys
import numpy as np

sys.path.insert(0, "/opt/trn_rl_repo")

import ml_dtypes
from contextlib import ExitStack

import concourse.bass as bass
import concourse.tile as tile
from concourse import bacc, mybir
from concourse.bass_utils import run_bass_kernel_spmd
from concourse.masks import make_identity

f32 = mybir.dt.float32
bf16 = mybir.dt.bfloat16
i32 = mybir.dt.int32
AF = mybir.ActivationFunctionType
ALU = mybir.AluOpType
bfnp = ml_dtypes.bfloat16

B, L, V, T, E, H = 128, 512, 30000, 20, 256, 256
NC_CORES = 8
BC = B // NC_CORES            # 16 sequences per core
GQ = 1                        # gather chunks per indirect-DMA call


def _pack_lstm_w(w_ih, w_hh, b_ih, b_hh, in_scale):
    Wcat = np.concatenate([w_ih.T * in_scale, w_hh.T * 0.5], axis=0).astype(np.float64)
    bias = (b_ih + b_hh).astype(np.float64)[None, :]
    M = np.concatenate([Wcat, bias], axis=0)
    M[:, 2 * H:3 * H] *= 2.0  # g-gate pre-double (ACT computes tanh(0.5*x))
    return M


def _build_host_inputs(x, tags, emb, w_ih0, w_hh0, b_ih0, b_hh0,
                       w_ih1, w_hh1, b_ih1, b_hh1, fc_W, fc_b,
                       crf_trans, crf_start, crf_end, Lsteps):
    ntb = Lsteps * BC
    nch = ntb // 128
    shared = {}
    # fp32 table: the indirect-DMA offset path quantizes indices through the
    # transfer dtype, so a bf16 table silently rounds indices >256.
    shared["emb_f"] = np.ascontiguousarray(emb.astype(np.float32))
    w0 = np.zeros((128, 2 * 5 * 1024), dtype=np.float64)
    bias0 = np.zeros((128, 16), dtype=np.float64)
    for d in range(2):
        M = _pack_lstm_w(w_ih0[d], w_hh0[d], b_ih0[d], b_hh0[d], 1.0)  # [513,1024]
        for k in range(4):
            w0[:, (d * 5 + k) * 1024:(d * 5 + k + 1) * 1024] = M[k * 128:(k + 1) * 128]
        w0[0, (d * 5 + 4) * 1024:(d * 5 + 5) * 1024] = M[512]
        for gc in range(8):
            bias0[:, d * 8 + gc] = M[-1, gc * 128:(gc + 1) * 128]
    shared["w0"] = w0.astype(bfnp)
    shared["bias0"] = bias0.astype(np.float32)
    w1 = np.zeros((128, 2 * 7 * 1024), dtype=np.float64)
    bias1 = np.zeros((128, 16), dtype=np.float64)
    for d in range(2):
        M = _pack_lstm_w(w_ih1[d], w_hh1[d], b_ih1[d], b_hh1[d], 0.5)  # [769,1024]
        for k in range(6):
            w1[:, (d * 7 + k) * 1024:(d * 7 + k + 1) * 1024] = M[k * 128:(k + 1) * 128]
        w1[0, (d * 7 + 6) * 1024:(d * 7 + 7) * 1024] = M[768]
        for gc in range(8):
            bias1[:, d * 8 + gc] = M[-1, gc * 128:(gc + 1) * 128]
    shared["w1"] = w1.astype(bfnp)
    shared["bias1"] = bias1.astype(np.float32)
    fcp = np.zeros((128, 4 * T), dtype=np.float64)
    fw = fc_W.T * 0.5
    for k in range(4):
        fcp[:, k * T:(k + 1) * T] = fw[k * 128:(k + 1) * 128]
    shared["fcp"] = fcp.astype(bfnp)
    shared["fcb3"] = (fc_b.astype(np.float64) - 3.0)[None, :].astype(np.float32)
    shared["mexp"] = np.exp(crf_trans.astype(np.float64)).astype(np.float32)
    shared["transb"] = crf_trans.astype(bfnp)
    shared["startexp"] = np.exp(crf_start.astype(np.float64)).astype(np.float32)[:, None]
    shared["startT"] = crf_start.astype(np.float32)[:, None]
    shared["endexp"] = np.exp(crf_end.astype(np.float64)).astype(np.float32)[:, None]
    shared["endT"] = crf_end.astype(np.float32)[:, None]
    shared["iota20"] = np.arange(T, dtype=np.float32)[:, None]

    per_core = []
    for c in range(NC_CORES):
        xc = x[c * BC:(c + 1) * BC, :Lsteps].astype(np.int64)
        tc_ = tags[c * BC:(c + 1) * BC, :Lsteps].astype(np.int64)
        flat = xc.T.reshape(-1).astype(np.int32)            # tb = t*BC + b
        xi = np.ascontiguousarray(flat.reshape(nch, 128).T)  # [128, nch]
        tgf = tc_.T.reshape(-1)
        per_core.append({"xi": xi, "tg": tgf.astype(bfnp)[None, :]})
    return shared, per_core


def _emit_lstm_layer(nc, pools, lyr, Lsteps, eT_tiles, wtile, wblk,
                     hT_f, hT_b, biasT, ident_bf):
    # Gate-major formulation: gates live on PARTITIONS (8 chunks of 128 per
    # dir), sequences on columns. The in-loop matmul keeps W_hh stationary
    # ([128f x 128g] blocks -> FWL) and streams hT (16 cols); h is produced
    # directly in hT layout so there is no per-step transpose.
    work, psum_g, psum_h, state, gxp = (pools["work"], pools["psum_g"],
                                        pools["psum_h"], pools["state"],
                                        pools["gx"])
    kE = len(eT_tiles)
    nblk = Lsteps * BC
    nwin = Lsteps // 8

    def produce(w):
        # Gx for window w (gate-major): out[g, tb] accumulated over kE
        # feature blocks; bias folds into the psum->SBUF ACT copy.
        out = []
        for d in (0, 1):
            c = w if d == 0 else nwin - 1 - w
            gxb = gxp.tile([128, 1024], bf16, tag=f"gx{d}")
            for gc in range(8):
                gp = psum_g.tile([128, 128], f32, tag="gxp")
                for k in range(kE):
                    et, blk = eT_tiles[k]
                    rhs = et[:, blk * nblk + c * 128: blk * nblk + (c + 1) * 128]
                    wcol = (wblk * d + k) * 1024 + gc * 128
                    nc.tensor.matmul(gp[:], wtile[:, wcol:wcol + 128], rhs,
                                     start=(k == 0), stop=(k == kE - 1))
                if gc % 2 == 0:
                    nc.scalar.activation(gxb[:, gc * 128:(gc + 1) * 128], gp[:],
                                         AF.Identity,
                                         bias=biasT[:, d * 8 + gc:d * 8 + gc + 1])
                else:
                    nc.vector.tensor_scalar(gxb[:, gc * 128:(gc + 1) * 128],
                                            gp[:], biasT[:, d * 8 + gc:d * 8 + gc + 1],
                                            None, op0=ALU.add)
            out.append(gxb)
        return out

    # 4 independent recurrence chains (2 dirs x 2 seq-groups of 8): chain
    # latencies hide each other on the shared engines. Each chain owns a
    # 64-col half of its dir's gate psum, laid out [gc 8][s 8] chain-major.
    C2 = {}
    for d in (0, 1):
        for sg in (0, 1):
            C2[(d, sg)] = state.tile([128, 16], f32, tag=f"C2_{lyr}_{d}{sg}",
                                     name=f"C2_{lyr}_{d}{sg}")
    gx = produce(0)
    for step in range(Lsteps):
        w, r = step // 8, step % 8
        if r == 0 and w + 1 < nwin:
            nxt = produce(w + 1)
        gps_d = {}
        for d in (0, 1):
            gps_d[d] = psum_h.tile([128, 128], f32, tag=f"g{d}", name=f"g{d}")
        for d in (0, 1):
            t_d = step if d == 0 else Lsteps - 1 - step
            rho = r if d == 0 else 7 - r
            ht = hT_f if d == 0 else hT_b
            for sg in (0, 1):
                gps = gps_d[d]
                c0 = sg * 64
                gxs = gx[d][:].rearrange("p (g n) -> p g n", g=8)[
                    :, :, rho * 16 + sg * 8: rho * 16 + sg * 8 + 8]
                T = work.tile([128, 64], f32, tag=f"T{d}{sg}", name=f"T{d}{sg}")
                if step > 0:
                    t_prev = t_d - 1 if d == 0 else t_d + 1
                    hcol = t_prev * BC + sg * 8
                    for gc in range(8):
                        for fb in (0, 1):
                            wcol = (wblk * d + kE + fb) * 1024 + gc * 128
                            nc.tensor.matmul(
                                gps[:, c0 + gc * 8: c0 + gc * 8 + 8],
                                wtile[:, wcol:wcol + 128],
                                ht[:, fb * nblk + hcol: fb * nblk + hcol + 8],
                                start=(fb == 0), stop=False)
                # identity matmul accumulates Gx into the gate psum
                nc.tensor.matmul(gps[:, c0:c0 + 64], ident_bf[:], gxs,
                                 start=(step == 0), stop=True)
                nc.scalar.activation(T[:], gps[:, c0:c0 + 64], AF.Tanh, scale=0.5)
                # chain cols: i 0:16 | f 16:32 | g 32:48 | o 48:64
                A = work.tile([128, 16], f32, tag=f"A{d}{sg}", name=f"A{d}{sg}")
                nc.vector.scalar_tensor_tensor(A[:], T[:, 0:16], 1.0, T[:, 32:48],
                                               op0=ALU.add, op1=ALU.mult)
                cc = C2[(d, sg)]
                if step > 0:
                    Bt = work.tile([128, 16], f32, tag=f"B{d}{sg}", name=f"B{d}{sg}")
                    nc.vector.scalar_tensor_tensor(Bt[:], T[:, 16:32], 1.0, cc[:],
                                                   op0=ALU.add, op1=ALU.mult)
                    nc.vector.scalar_tensor_tensor(cc[:], Bt[:], 0.5, A[:],
                                                   op0=ALU.mult, op1=ALU.add)
                else:
                    nc.vector.tensor_copy(cc[:], A[:])
                TC = work.tile([128, 16], f32, tag=f"TC{d}{sg}", name=f"TC{d}{sg}")
                nc.scalar.activation(TC[:], cc[:], AF.Tanh, scale=0.5)
                dst = ht[:].rearrange("p (k n) -> p k n", k=2, n=nblk)[
                    :, :, t_d * BC + sg * 8: t_d * BC + sg * 8 + 8]
                nc.vector.scalar_tensor_tensor(
                    dst, T[:, 48:64].rearrange("p (k n) -> p k n", k=2, n=8), 1.0,
                    TC[:].rearrange("p (k n) -> p k n", k=2, n=8),
                    op0=ALU.add, op1=ALU.mult)
        if r == 7 and w + 1 < nwin:
            gx = nxt


def build_nc(Lsteps=L, debug_outs=()):
    nc = bacc.Bacc("TRN2", target_bir_lowering=False, debug=False)
    ntb = Lsteps * BC
    nch = ntb // 128
    dp = lambda n, s, dt: nc.declare_dram_parameter(n, s, dt, isOutput=False).ap()
    xi_i = dp("xi", [128, nch], i32)
    tg_i = dp("tg", [1, ntb], bf16)
    emb_i = dp("emb_f", [V, E], f32)
    w0_i = dp("w0", [128, 10240], bf16)
    w1_i = dp("w1", [128, 14336], bf16)
    bias0_i = dp("bias0", [128, 16], f32)
    bias1_i = dp("bias1", [128, 16], f32)
    fcp_i = dp("fcp", [128, 4 * T], bf16)
    fcb3_i = dp("fcb3", [1, T], f32)
    mexp_i = dp("mexp", [T, T], f32)
    transb_i = dp("transb", [T, T], bf16)
    startexp_i = dp("startexp", [T, 1], f32)
    startT_i = dp("startT", [T, 1], f32)
    endexp_i = dp("endexp", [T, 1], f32)
    endT_i = dp("endT", [T, 1], f32)
    iota_i = dp("iota20", [T, 1], f32)
    v_o = nc.declare_dram_parameter("v", [1, BC], f32, isOutput=True).ap()
    dbg = {}
    if "h0f" in debug_outs:
        for nm, sh, dt in (("h0f", [128, 2 * ntb], bf16), ("h0b", [128, 2 * ntb], bf16),
                           ("h1f", [128, 2 * ntb], bf16), ("h1b", [128, 2 * ntb], bf16),
                           ("eTo", [128, 2 * ntb], bf16)):
            dbg[nm] = nc.declare_dram_parameter(nm, sh, dt, isOutput=True).ap()
    if "score" in debug_outs:
        dbg["score"] = nc.declare_dram_parameter("score", [1, BC], f32, isOutput=True).ap()
        dbg["SL"] = nc.declare_dram_parameter("SL", [T, BC], f32, isOutput=True).ap()
        dbg["expem"] = nc.declare_dram_parameter("expem", [T, ntb], bf16, isOutput=True).ap()

    with tile.TileContext(nc) as tc, ExitStack() as ctx:
        consts = ctx.enter_context(tc.tile_pool(name="consts", bufs=1))
        wpool = ctx.enter_context(tc.tile_pool(name="wpool", bufs=1))
        slotA = ctx.enter_context(tc.tile_pool(name="slotA", bufs=1))
        hbuf = ctx.enter_context(tc.tile_pool(name="hbuf", bufs=1))
        state = ctx.enter_context(tc.tile_pool(name="state", bufs=1))
        work = ctx.enter_context(tc.tile_pool(name="work", bufs=2))
        stage = ctx.enter_context(tc.tile_pool(name="stage", bufs=2))
        psum_g = ctx.enter_context(tc.tile_pool(name="psum_g", bufs=2, space="PSUM"))
        psum_h = ctx.enter_context(tc.tile_pool(name="psum_h", bufs=1, space="PSUM"))
        gxpool = ctx.enter_context(tc.tile_pool(name="gx", bufs=2))
        psum_e = psum_g
        psum_s = psum_g
        pools = dict(work=work, psum_g=psum_g, psum_h=psum_h, state=state,
                     gx=gxpool)

        ident_bf = consts.tile([128, 128], bf16)
        make_identity(nc, ident_bf)
        ones512f = consts.tile([1, 512], f32)
        nc.vector.memset(ones512f[:], 1.0)
        ones20f = consts.tile([T, 1], f32)
        nc.vector.memset(ones20f[:], 1.0)
        ones1_20 = consts.tile([1, T], bf16)
        nc.vector.memset(ones1_20[:], 1.0)

        def cload(name, src, shape, dt):
            t = consts.tile(shape, dt, tag=name)
            nc.sync.dma_start(t[:], src[:])
            return t
        mexp = cload("mexp", mexp_i, [T, T], f32)
        transb = cload("transb", transb_i, [T, T], bf16)
        startexp = cload("startexp", startexp_i, [T, 1], f32)
        startT = cload("startT", startT_i, [T, 1], f32)
        endexp = cload("endexp", endexp_i, [T, 1], f32)
        endT = cload("endT", endT_i, [T, 1], f32)
        iota20 = cload("iota20", iota_i, [T, 1], f32)
        fcb3 = cload("fcb3", fcb3_i, [1, T], f32)
        fcp = cload("fcp", fcp_i, [128, 4 * T], bf16)
        idx = cload("idx", xi_i, [128, nch], i32)
        bias0 = cload("bias0", bias0_i, [128, 16], f32)
        bias1 = cload("bias1", bias1_i, [128, 16], f32)

        w0 = wpool.tile([128, 14336], bf16, tag="wslot")

        # ---------- P0: embedding gather + transpose ----------
        nc.sync.dma_start(w0[:, 0:10240], w0_i[:])
        eT = slotA.tile([128, 2 * ntb], bf16, tag="slotA")
        for c in range(nch):
            st = stage.tile([128, E], f32, tag="gstage")
            nc.gpsimd.indirect_dma_start(
                out=st[:], out_offset=None, in_=emb_i[:],
                in_offset=bass.IndirectOffsetOnAxis(ap=idx[:, c:c + 1], axis=0))
            stb = stage.tile([128, E], bf16, tag="gconv")
            nc.vector.tensor_copy(stb[:], st[:])
            eps = psum_g.tile([128, 2 * 128], bf16, tag="gxp")
            nc.tensor.transpose(eps[:, 0:128], stb[:, 0:128], ident_bf[:])
            nc.tensor.transpose(eps[:, 128:256], stb[:, 128:256], ident_bf[:])
            dst = eT[:].rearrange("p (k n) -> p k n", k=2, n=ntb)[:, :, c * 128:(c + 1) * 128]
            nc.vector.tensor_copy(dst, eps[:].rearrange("p (k c) -> p k c", k=2))

        # ---------- P1: layer 0 ----------
        h0f = hbuf.tile([128, 2 * ntb], bf16, tag="h0f")
        h0b = hbuf.tile([128, 2 * ntb], bf16, tag="h0b")
        _emit_lstm_layer(nc, pools, 0, Lsteps, [(eT, 0), (eT, 1)], w0, 5,
                         h0f, h0b, bias0, ident_bf)

        # ---------- P2: layer 1 ----------
        w1 = wpool.tile([128, 14336], bf16, tag="wslot")
        nc.sync.dma_start(w1[:], w1_i[:])
        h1f = slotA.tile([128, 2 * ntb], bf16, tag="slotA")
        h1b = hbuf.tile([128, 2 * ntb], bf16, tag="h1b")
        _emit_lstm_layer(nc, pools, 1, Lsteps,
                         [(h0f, 0), (h0f, 1), (h0b, 0), (h0b, 1)], w1, 7,
                         h1f, h1b, bias1, ident_bf)
        if "h0f" in dbg:
            nc.sync.dma_start(dbg["eTo"][:], eT[:])
            nc.sync.dma_start(dbg["h0f"][:], h0f[:])
            nc.sync.dma_start(dbg["h0b"][:], h0b[:])
            nc.sync.dma_start(dbg["h1f"][:], h1f[:])
            nc.sync.dma_start(dbg["h1b"][:], h1b[:])

        # ---------- P3+P4: FC, expem, one-hot, score pieces (chunked) ----------
        # expem lives in DRAM scratch (16KB/partition of SBUF saved); the
        # P5 scan prefetches it back chunk-by-chunk
        expem_d = nc.dram_tensor("expem_d", [T, ntb], bf16).ap()
        pile = state.tile([T, BC], f32, tag="pile")
        nc.vector.memset(pile[:], 0.0)
        red = work.tile([T, BC], f32, tag="red")
        ncol = 512
        nchunks = (ntb + ncol - 1) // ncol
        for ci in range(nchunks):
            n0 = ci * ncol
            nn = min(ncol, ntb - n0)
            nt = nn // BC
            em_ps = psum_e.tile([T, ncol], f32, tag="gates")
            nc.tensor.matmul(em_ps[:, 0:nn], fcb3[:], ones512f[:, 0:nn],
                             start=True, stop=False)
            for k in range(4):
                ht = h1f if k < 2 else h1b
                kk = k % 2
                nc.tensor.matmul(em_ps[:, 0:nn], fcp[:, k * T:(k + 1) * T],
                                 ht[:, kk * ntb + n0: kk * ntb + n0 + nn],
                                 start=False, stop=(k == 3))
            exc = stage.tile([T, ncol], bf16, tag="exc")
            nc.scalar.activation(exc[:, 0:nn], em_ps[:, 0:nn], AF.Exp)
            nc.sync.dma_start(expem_d[:, n0:n0 + nn], exc[:, 0:nn])
            if "expem" in dbg:
                nc.sync.dma_start(dbg["expem"][:, n0:n0 + nn], exc[:, 0:nn])
            # one-hot of tags for this chunk (+16-shifted variant for transitions)
            tgc = stage.tile([1, ncol + BC], bf16, tag="tgc")
            nsh = min(nn + BC, ntb - n0)
            nc.sync.dma_start(tgc[:, 0:nsh], tg_i[:, n0:n0 + nsh])
            tg_ps = psum_s.tile([T, ncol], f32, tag="gates")
            nc.tensor.matmul(tg_ps[:, 0:nn], ones1_20[:], tgc[:, 0:nn],
                             start=True, stop=True)
            M1c = work.tile([T, ncol + BC], bf16, tag="M1c")
            nc.vector.tensor_scalar(M1c[:, 0:nn], tg_ps[:, 0:nn], iota20[:], None,
                                    op0=ALU.is_equal)
            if nsh > nn:
                tg_ps2 = psum_s.tile([T, ncol], f32, tag="gates")
                nc.tensor.matmul(tg_ps2[:, 0:nsh - nn], ones1_20[:], tgc[:, nn:nsh],
                                 start=True, stop=True)
                nc.vector.tensor_scalar(M1c[:, nn:nsh], tg_ps2[:, 0:nsh - nn],
                                        iota20[:], None, op0=ALU.is_equal)
            # emission part of score
            EMSc = work.tile([T, ncol], f32, tag="EMSc")
            nc.vector.tensor_mul(EMSc[:, 0:nn], em_ps[:, 0:nn], M1c[:, 0:nn])
            nc.vector.reduce_sum(red[:],
                                 EMSc[:, 0:nn].rearrange("p (t b) -> p b t", b=BC),
                                 axis=mybir.AxisListType.X)
            nc.vector.tensor_add(pile[:], pile[:], red[:])
            # transition part: G[:,tb]*M1[:,tb+BC], tb in [n0, n0+nn-?)
            g_ps = psum_s.tile([T, ncol], f32, tag="gates")
            nc.tensor.matmul(g_ps[:, 0:nn], transb[:], M1c[:, 0:nn],
                             start=True, stop=True)
            n3 = nn if n0 + nn < ntb else nn - BC
            if n3 > 0:
                S3c = work.tile([T, ncol], f32, tag="EMSc")
                nc.vector.tensor_mul(S3c[:, 0:n3], g_ps[:, 0:n3], M1c[:, BC:BC + n3])
                nc.vector.reduce_sum(red[:],
                                     S3c[:, 0:n3].rearrange("p (t b) -> p b t", b=BC),
                                     axis=mybir.AxisListType.X)
                nc.vector.tensor_add(pile[:], pile[:], red[:])
            # start / end parts
            if ci == 0:
                nc.vector.tensor_scalar(red[:], M1c[:, 0:BC], startT[:], None,
                                        op0=ALU.mult)
                nc.vector.tensor_add(pile[:], pile[:], red[:])
            if n0 + nn == ntb:
                nc.vector.tensor_scalar(red[:], M1c[:, nn - BC:nn], endT[:], None,
                                        op0=ALU.mult)
                nc.vector.tensor_add(pile[:], pile[:], red[:])
        sc_ps = psum_s.tile([1, BC], f32, tag="small")
        nc.tensor.matmul(sc_ps[:], ones20f[:], pile[:], start=True, stop=True)
        score = state.tile([1, BC], f32, tag="score")
        nc.vector.tensor_copy(score[:], sc_ps[:])
        if "score" in dbg:
            nc.sync.dma_start(dbg["score"][:], score[:])

        # ---------- P5: exp-space forward scan ----------
        STEPS_PER_ECH = ncol // BC        # 32 scan steps per expem chunk

        def loadch(k):
            t_ = stage.tile([T, ncol], bf16, tag="exc")
            nc.sync.dma_start(t_[:], expem_d[:, k * ncol:(k + 1) * ncol])
            return t_

        ex_cur = loadch(0)
        ex_nxt = loadch(1) if nchunks > 1 else None
        Scur = state.tile([T, BC], f32, tag="S0")
        nc.vector.tensor_scalar(Scur[:], ex_cur[:, 0:BC], startexp[:], None,
                                op0=ALU.mult)
        for t in range(1, Lsteps):
            k, off = divmod(t, STEPS_PER_ECH)
            if off == 0:
                ex_cur = ex_nxt
                if k + 1 < nchunks:
                    ex_nxt = loadch(k + 1)
            sp = psum_s.tile([T, BC], f32, tag="small")
            nc.tensor.matmul(sp[:], mexp[:], Scur[:], start=True, stop=True)
            Snew = work.tile([T, BC], f32, tag="Sflip")
            nc.vector.tensor_mul(Snew[:], sp[:], ex_cur[:, off * BC:(off + 1) * BC])
            Scur = Snew
        if "score" in dbg:
            nc.sync.dma_start(dbg["SL"][:], Scur[:])
        EE = state.tile([T, BC], f32, tag="EE")
        nc.vector.tensor_scalar(EE[:], Scur[:], endexp[:], None, op0=ALU.mult)
        z_ps = psum_s.tile([1, BC], f32, tag="small")
        nc.tensor.matmul(z_ps[:], ones20f[:], EE[:], start=True, stop=True)
        vt = state.tile([1, BC], f32, tag="vt")
        nc.scalar.activation(vt[:], z_ps[:], AF.Ln)
        nc.vector.tensor_sub(vt[:], vt[:], score[:])
        nc.sync.dma_start(v_o[:], vt[:])
    nc.compile()
    return nc


def _host_inputs_from_dict(np_in, Lsteps):
    f = lambda k: np.asarray(np_in[k], np.float32)
    return _build_host_inputs(
        np.asarray(np_in["x"]), np.asarray(np_in["tags"]), f("emb"),
        f("w_ih0"), f("w_hh0"), f("b_ih0"), f("b_hh0"),
        f("w_ih1"), f("w_hh1"), f("b_ih1"), f("b_hh1"),
        f("fc_W"), f("fc_b"), f("crf_trans"), f("crf_start"), f("crf_end"),
        Lsteps)


TRACE = False          # set by test harnesses to capture an NTFF profile
LAST_RESULTS = None


def kernel(**inputs):
    global LAST_RESULTS
    np_in = {k: np.asarray(v) for k, v in inputs.items()}
    shared, per_core = _host_inputs_from_dict(np_in, L)
    nc = build_nc(L)
    in_maps = [dict(shared, **pc) for pc in per_core]
    LAST_RESULTS = run_bass_kernel_spmd(nc, in_maps, list(range(NC_CORES)),
                                        trace=TRACE)
    v = np.concatenate([r["v"][0] for r in LAST_RESULTS.results])
    return np.float32(np.mean(v))



# revision 26
# speedup vs baseline: 1.1428x; 1.1428x over previous
# BiLSTM-CRF NLL kernel for 8x Trainium2 NeuronCores (Bass/Tile).
#
# Strategy: data-parallel over batch (16 seqs/core). Per core:
#   P0  embedding gather (indirect DMA) + PE-transpose -> eT [feat, time*batch] bf16
#   P1  BiLSTM layer 0: fused fwd+bwd step pipeline; strip-layout gates in one
#       PSUM bank via 4 tensor-engine column groups; single-func activation
#       (sigmoid(x) = (tanh(x/2)+1)/2, tanh-gate columns pre-doubled host-side);
#       cell state kept as C2=2c, hidden stored as Hh=2h (downstream weights
#       pre-halved host-side); per-step PE transpose of Hh -> hT storage.
#   P2  BiLSTM layer 1 (input = layer-0 output)
#   P3+P4  FC -> emissions em3 = em - 3 (drift fold); fused per-chunk:
#       expem = exp(em3) (bf16, kept), one-hot M1 from tags, gold-path score
#       pieces accumulated via strided reduces + matmuls.
#   P5  CRF partition function in pure exp space:
#         S_t = (exp(trans)^T @ S_{t-1}) * expem_t   (no ACT in the scan loop)
#       logZ_dev = ln(sum_j S_L * exp(end));  v_b = logZ_dev - score_dev
#       (the -3/step drift cancels exactly between logZ_dev and score_dev)
# Host: output = mean over all 128 v_b.
import sys
import numpy as np

sys.path.insert(0, "/opt/trn_rl_repo")

import ml_dtypes
from contextlib import ExitStack

import concourse.bass as bass
import concourse.tile as tile
from concourse import bacc, mybir
from concourse.bass_utils import run_bass_kernel_spmd
from concourse.masks import make_identity

f32 = mybir.dt.float32
bf16 = mybir.dt.bfloat16
i32 = mybir.dt.int32
AF = mybir.ActivationFunctionType
ALU = mybir.AluOpType
bfnp = ml_dtypes.bfloat16

B, L, V, T, E, H = 128, 512, 30000, 20, 256, 256
NC_CORES = 8
BC = B // NC_CORES            # 16 sequences per core
GQ = 1                        # gather chunks per indirect-DMA call


def _pack_lstm_w(w_ih, w_hh, b_ih, b_hh, in_scale):
    Wcat = np.concatenate([w_ih.T * in_scale, w_hh.T * 0.5], axis=0).astype(np.float64)
    bias = (b_ih + b_hh).astype(np.float64)[None, :]
    M = np.concatenate([Wcat, bias], axis=0)
    M[:, 2 * H:3 * H] *= 2.0  # g-gate pre-double (ACT computes tanh(0.5*x))
    return M


def _build_host_inputs(x, tags, emb, w_ih0, w_hh0, b_ih0, b_hh0,
                       w_ih1, w_hh1, b_ih1, b_hh1, fc_W, fc_b,
                       crf_trans, crf_start, crf_end, Lsteps):
    ntb = Lsteps * BC
    nch = ntb // 128
    shared = {}
    # fp32 table: the indirect-DMA offset path quantizes indices through the
    # transfer dtype, so a bf16 table silently rounds indices >256.
    shared["emb_f"] = np.ascontiguousarray(emb.astype(np.float32))
    w0 = np.zeros((128, 2 * 5 * 1024), dtype=np.float64)
    bias0 = np.zeros((128, 16), dtype=np.float64)
    for d in range(2):
        M = _pack_lstm_w(w_ih0[d], w_hh0[d], b_ih0[d], b_hh0[d], 1.0)  # [513,1024]
        for k in range(4):
            w0[:, (d * 5 + k) * 1024:(d * 5 + k + 1) * 1024] = M[k * 128:(k + 1) * 128]
        w0[0, (d * 5 + 4) * 1024:(d * 5 + 5) * 1024] = M[512]
        for gc in range(8):
            bias0[:, d * 8 + gc] = M[-1, gc * 128:(gc + 1) * 128]
    shared["w0"] = w0.astype(bfnp)
    shared["bias0"] = bias0.astype(np.float32)
    w1 = np.zeros((128, 2 * 7 * 1024), dtype=np.float64)
    bias1 = np.zeros((128, 16), dtype=np.float64)
    for d in range(2):
        M = _pack_lstm_w(w_ih1[d], w_hh1[d], b_ih1[d], b_hh1[d], 0.5)  # [769,1024]
        for k in range(6):
            w1[:, (d * 7 + k) * 1024:(d * 7 + k + 1) * 1024] = M[k * 128:(k + 1) * 128]
        w1[0, (d * 7 + 6) * 1024:(d * 7 + 7) * 1024] = M[768]
        for gc in range(8):
            bias1[:, d * 8 + gc] = M[-1, gc * 128:(gc + 1) * 128]
    shared["w1"] = w1.astype(bfnp)
    shared["bias1"] = bias1.astype(np.float32)
    fcp = np.zeros((128, 4 * T), dtype=np.float64)
    fw = fc_W.T * 0.5
    for k in range(4):
        fcp[:, k * T:(k + 1) * T] = fw[k * 128:(k + 1) * 128]
    shared["fcp"] = fcp.astype(bfnp)
    shared["fcb3"] = (fc_b.astype(np.float64) - 3.0)[None, :].astype(np.float32)
    shared["mexp"] = np.exp(crf_trans.astype(np.float64)).astype(np.float32)
    shared["transb"] = crf_trans.astype(bfnp)
    shared["startexp"] = np.exp(crf_start.astype(np.float64)).astype(np.float32)[:, None]
    shared["startT"] = crf_start.astype(np.float32)[:, None]
    shared["endexp"] = np.exp(crf_end.astype(np.float64)).astype(np.float32)[:, None]
    shared["endT"] = crf_end.astype(np.float32)[:, None]
    shared["iota20"] = np.arange(T, dtype=np.float32)[:, None]

    per_core = []
    for c in range(NC_CORES):
        xc = x[c * BC:(c + 1) * BC, :Lsteps].astype(np.int64)
        tc_ = tags[c * BC:(c + 1) * BC, :Lsteps].astype(np.int64)
        flat = xc.T.reshape(-1).astype(np.int32)            # tb = t*BC + b
        xi = np.ascontiguousarray(flat.reshape(nch, 128).T)  # [128, nch]
        tgf = tc_.T.reshape(-1)
        per_core.append({"xi": xi, "tg": tgf.astype(bfnp)[None, :]})
    return shared, per_core


def _emit_lstm_layer(nc, pools, lyr, Lsteps, eT_tiles, wtile, wblk,
                     hT_f, hT_b, biasT, ident_bf):
    # Gate-major formulation: gates live on PARTITIONS (8 chunks of 128 per
    # dir), sequences on columns. The in-loop matmul keeps W_hh stationary
    # ([128f x 128g] blocks -> FWL) and streams hT (16 cols); h is produced
    # directly in hT layout so there is no per-step transpose.
    work, psum_g, psum_h, state, gxp = (pools["work"], pools["psum_g"],
                                        pools["psum_h"], pools["state"],
                                        pools["gx"])
    kE = len(eT_tiles)
    nblk = Lsteps * BC
    nwin = Lsteps // 8

    def produce(w):
        # Gx for window w (gate-major): out[g, tb] accumulated over kE
        # feature blocks; bias folds into the psum->SBUF ACT copy.
        out = []
        for d in (0, 1):
            c = w if d == 0 else nwin - 1 - w
            gxb = gxp.tile([128, 1024], bf16, tag=f"gx{d}")
            for gc in range(8):
                gp = psum_g.tile([128, 128], f32, tag="gxp")
                for k in range(kE):
                    et, blk = eT_tiles[k]
                    rhs = et[:, blk * nblk + c * 128: blk * nblk + (c + 1) * 128]
                    wcol = (wblk * d + k) * 1024 + gc * 128
                    nc.tensor.matmul(gp[:], wtile[:, wcol:wcol + 128], rhs,
                                     start=(k == 0), stop=(k == kE - 1))
                if gc % 2 == 0:
                    nc.scalar.activation(gxb[:, gc * 128:(gc + 1) * 128], gp[:],
                                         AF.Identity,
                                         bias=biasT[:, d * 8 + gc:d * 8 + gc + 1])
                else:
                    nc.vector.tensor_scalar(gxb[:, gc * 128:(gc + 1) * 128],
                                            gp[:], biasT[:, d * 8 + gc:d * 8 + gc + 1],
                                            None, op0=ALU.add)
            out.append(gxb)
        return out

    C2 = [state.tile([128, 32], f32, tag=f"C2_{lyr}_{d}", name=f"C2_{lyr}_{d}")
          for d in (0, 1)]
    gx = produce(0)
    for step in range(Lsteps):
        w, r = step // 8, step % 8
        if r == 0 and w + 1 < nwin:
            nxt = produce(w + 1)
        for d in (0, 1):
            t_d = step if d == 0 else Lsteps - 1 - step
            rho = r if d == 0 else 7 - r
            ht = hT_f if d == 0 else hT_b
            gxs = gx[d][:].rearrange("p (g n) -> p g n", g=8)[:, :, rho * 16:rho * 16 + 16]
            T = work.tile([128, 128], f32, tag=f"T{d}", name=f"T{d}")
            gps = psum_h.tile([128, 128], f32, tag=f"g{d}", name=f"g{d}")
            if step > 0:
                t_prev = t_d - 1 if d == 0 else t_d + 1
                for gc in range(8):
                    for fb in (0, 1):
                        wcol = (wblk * d + kE + fb) * 1024 + gc * 128
                        nc.tensor.matmul(
                            gps[:, gc * 16:gc * 16 + 16],
                            wtile[:, wcol:wcol + 128],
                            ht[:, fb * nblk + t_prev * BC: fb * nblk + (t_prev + 1) * BC],
                            start=(fb == 0), stop=(fb == 1))
                nc.vector.tensor_add(T[:], gps[:], gxs)
                nc.scalar.activation(T[:], T[:], AF.Tanh, scale=0.5)
            else:
                nc.scalar.activation(T[:], gxs, AF.Tanh, scale=0.5)
            # cols: i 0:32 | f 32:64 | g 64:96 | o 96:128 (16 per fblock)
            A = work.tile([128, 32], f32, tag=f"A{d}", name=f"A{d}")
            nc.vector.scalar_tensor_tensor(A[:], T[:, 0:32], 1.0, T[:, 64:96],
                                           op0=ALU.add, op1=ALU.mult)
            if step > 0:
                Bt = work.tile([128, 32], f32, tag=f"B{d}", name=f"B{d}")
                nc.vector.scalar_tensor_tensor(Bt[:], T[:, 32:64], 1.0, C2[d][:],
                                               op0=ALU.add, op1=ALU.mult)
                nc.vector.scalar_tensor_tensor(C2[d][:], Bt[:], 0.5, A[:],
                                               op0=ALU.mult, op1=ALU.add)
            else:
                nc.vector.tensor_copy(C2[d][:], A[:])
            TC = work.tile([128, 32], f32, tag=f"TC{d}", name=f"TC{d}")
            nc.scalar.activation(TC[:], C2[d][:], AF.Tanh, scale=0.5)
            dst = ht[:].rearrange("p (k n) -> p k n", k=2, n=nblk)[:, :, t_d * BC:(t_d + 1) * BC]
            nc.vector.scalar_tensor_tensor(dst, T[:, 96:128], 1.0, TC[:],
                                           op0=ALU.add, op1=ALU.mult)
        if r == 7 and w + 1 < nwin:
            gx = nxt


def build_nc(Lsteps=L, debug_outs=()):
    nc = bacc.Bacc("TRN2", target_bir_lowering=False, debug=False)
    ntb = Lsteps * BC
    nch = ntb // 128
    dp = lambda n, s, dt: nc.declare_dram_parameter(n, s, dt, isOutput=False).ap()
    xi_i = dp("xi", [128, nch], i32)
    tg_i = dp("tg", [1, ntb], bf16)
    emb_i = dp("emb_f", [V, E], f32)
    w0_i = dp("w0", [128, 10240], bf16)
    w1_i = dp("w1", [128, 14336], bf16)
    bias0_i = dp("bias0", [128, 16], f32)
    bias1_i = dp("bias1", [128, 16], f32)
    fcp_i = dp("fcp", [128, 4 * T], bf16)
    fcb3_i = dp("fcb3", [1, T], f32)
    mexp_i = dp("mexp", [T, T], f32)
    transb_i = dp("transb", [T, T], bf16)
    startexp_i = dp("startexp", [T, 1], f32)
    startT_i = dp("startT", [T, 1], f32)
    endexp_i = dp("endexp", [T, 1], f32)
    endT_i = dp("endT", [T, 1], f32)
    iota_i = dp("iota20", [T, 1], f32)
    v_o = nc.declare_dram_parameter("v", [1, BC], f32, isOutput=True).ap()
    dbg = {}
    if "h0f" in debug_outs:
        for nm, sh, dt in (("h0f", [128, 2 * ntb], bf16), ("h0b", [128, 2 * ntb], bf16),
                           ("h1f", [128, 2 * ntb], bf16), ("h1b", [128, 2 * ntb], bf16),
                           ("eTo", [128, 2 * ntb], bf16)):
            dbg[nm] = nc.declare_dram_parameter(nm, sh, dt, isOutput=True).ap()
    if "score" in debug_outs:
        dbg["score"] = nc.declare_dram_parameter("score", [1, BC], f32, isOutput=True).ap()
        dbg["SL"] = nc.declare_dram_parameter("SL", [T, BC], f32, isOutput=True).ap()
        dbg["expem"] = nc.declare_dram_parameter("expem", [T, ntb], bf16, isOutput=True).ap()

    with tile.TileContext(nc) as tc, ExitStack() as ctx:
        consts = ctx.enter_context(tc.tile_pool(name="consts", bufs=1))
        wpool = ctx.enter_context(tc.tile_pool(name="wpool", bufs=1))
        slotA = ctx.enter_context(tc.tile_pool(name="slotA", bufs=1))
        hbuf = ctx.enter_context(tc.tile_pool(name="hbuf", bufs=1))
        state = ctx.enter_context(tc.tile_pool(name="state", bufs=1))
        work = ctx.enter_context(tc.tile_pool(name="work", bufs=2))
        stage = ctx.enter_context(tc.tile_pool(name="stage", bufs=2))
        psum_g = ctx.enter_context(tc.tile_pool(name="psum_g", bufs=2, space="PSUM"))
        psum_h = ctx.enter_context(tc.tile_pool(name="psum_h", bufs=1, space="PSUM"))
        gxpool = ctx.enter_context(tc.tile_pool(name="gx", bufs=2))
        psum_e = psum_g
        psum_s = psum_g
        pools = dict(work=work, psum_g=psum_g, psum_h=psum_h, state=state,
                     gx=gxpool)

        ident_bf = consts.tile([128, 128], bf16)
        make_identity(nc, ident_bf)
        ones512f = consts.tile([1, 512], f32)
        nc.vector.memset(ones512f[:], 1.0)
        ones20f = consts.tile([T, 1], f32)
        nc.vector.memset(ones20f[:], 1.0)
        ones1_20 = consts.tile([1, T], bf16)
        nc.vector.memset(ones1_20[:], 1.0)

        def cload(name, src, shape, dt):
            t = consts.tile(shape, dt, tag=name)
            nc.sync.dma_start(t[:], src[:])
            return t
        mexp = cload("mexp", mexp_i, [T, T], f32)
        transb = cload("transb", transb_i, [T, T], bf16)
        startexp = cload("startexp", startexp_i, [T, 1], f32)
        startT = cload("startT", startT_i, [T, 1], f32)
        endexp = cload("endexp", endexp_i, [T, 1], f32)
        endT = cload("endT", endT_i, [T, 1], f32)
        iota20 = cload("iota20", iota_i, [T, 1], f32)
        fcb3 = cload("fcb3", fcb3_i, [1, T], f32)
        fcp = cload("fcp", fcp_i, [128, 4 * T], bf16)
        idx = cload("idx", xi_i, [128, nch], i32)
        bias0 = cload("bias0", bias0_i, [128, 16], f32)
        bias1 = cload("bias1", bias1_i, [128, 16], f32)

        w0 = wpool.tile([128, 14336], bf16, tag="wslot")

        # ---------- P0: embedding gather + transpose ----------
        nc.sync.dma_start(w0[:, 0:10240], w0_i[:])
        eT = slotA.tile([128, 2 * ntb], bf16, tag="slotA")
        for c in range(nch):
            st = stage.tile([128, E], f32, tag="gstage")
            nc.gpsimd.indirect_dma_start(
                out=st[:], out_offset=None, in_=emb_i[:],
                in_offset=bass.IndirectOffsetOnAxis(ap=idx[:, c:c + 1], axis=0))
            stb = stage.tile([128, E], bf16, tag="gconv")
            nc.vector.tensor_copy(stb[:], st[:])
            eps = psum_g.tile([128, 2 * 128], bf16, tag="gxp")
            nc.tensor.transpose(eps[:, 0:128], stb[:, 0:128], ident_bf[:])
            nc.tensor.transpose(eps[:, 128:256], stb[:, 128:256], ident_bf[:])
            dst = eT[:].rearrange("p (k n) -> p k n", k=2, n=ntb)[:, :, c * 128:(c + 1) * 128]
            nc.vector.tensor_copy(dst, eps[:].rearrange("p (k c) -> p k c", k=2))

        # ---------- P1: layer 0 ----------
        h0f = hbuf.tile([128, 2 * ntb], bf16, tag="h0f")
        h0b = hbuf.tile([128, 2 * ntb], bf16, tag="h0b")
        _emit_lstm_layer(nc, pools, 0, Lsteps, [(eT, 0), (eT, 1)], w0, 5,
                         h0f, h0b, bias0, ident_bf)

        # ---------- P2: layer 1 ----------
        w1 = wpool.tile([128, 14336], bf16, tag="wslot")
        nc.sync.dma_start(w1[:], w1_i[:])
        h1f = slotA.tile([128, 2 * ntb], bf16, tag="slotA")
        h1b = hbuf.tile([128, 2 * ntb], bf16, tag="h1b")
        _emit_lstm_layer(nc, pools, 1, Lsteps,
                         [(h0f, 0), (h0f, 1), (h0b, 0), (h0b, 1)], w1, 7,
                         h1f, h1b, bias1, ident_bf)
        if "h0f" in dbg:
            nc.sync.dma_start(dbg["eTo"][:], eT[:])
            nc.sync.dma_start(dbg["h0f"][:], h0f[:])
            nc.sync.dma_start(dbg["h0b"][:], h0b[:])
            nc.sync.dma_start(dbg["h1f"][:], h1f[:])
            nc.sync.dma_start(dbg["h1b"][:], h1b[:])

        # ---------- P3+P4: FC, expem, one-hot, score pieces (chunked) ----------
        # expem lives in DRAM scratch (16KB/partition of SBUF saved); the
        # P5 scan prefetches it back chunk-by-chunk
        expem_d = nc.dram_tensor("expem_d", [T, ntb], bf16).ap()
        pile = state.tile([T, BC], f32, tag="pile")
        nc.vector.memset(pile[:], 0.0)
        red = work.tile([T, BC], f32, tag="red")
        ncol = 512
        nchunks = (ntb + ncol - 1) // ncol
        for ci in range(nchunks):
            n0 = ci * ncol
            nn = min(ncol, ntb - n0)
            nt = nn // BC
            em_ps = psum_e.tile([T, ncol], f32, tag="gates")
            nc.tensor.matmul(em_ps[:, 0:nn], fcb3[:], ones512f[:, 0:nn],
                             start=True, stop=False)
            for k in range(4):
                ht = h1f if k < 2 else h1b
                kk = k % 2
                nc.tensor.matmul(em_ps[:, 0:nn], fcp[:, k * T:(k + 1) * T],
                                 ht[:, kk * ntb + n0: kk * ntb + n0 + nn],
                                 start=False, stop=(k == 3))
            exc = stage.tile([T, ncol], bf16, tag="exc")
            nc.scalar.activation(exc[:, 0:nn], em_ps[:, 0:nn], AF.Exp)
            nc.sync.dma_start(expem_d[:, n0:n0 + nn], exc[:, 0:nn])
            if "expem" in dbg:
                nc.sync.dma_start(dbg["expem"][:, n0:n0 + nn], exc[:, 0:nn])
            # one-hot of tags for this chunk (+16-shifted variant for transitions)
            tgc = stage.tile([1, ncol + BC], bf16, tag="tgc")
            nsh = min(nn + BC, ntb - n0)
            nc.sync.dma_start(tgc[:, 0:nsh], tg_i[:, n0:n0 + nsh])
            tg_ps = psum_s.tile([T, ncol], f32, tag="gates")
            nc.tensor.matmul(tg_ps[:, 0:nn], ones1_20[:], tgc[:, 0:nn],
                             start=True, stop=True)
            M1c = work.tile([T, ncol + BC], bf16, tag="M1c")
            nc.vector.tensor_scalar(M1c[:, 0:nn], tg_ps[:, 0:nn], iota20[:], None,
                                    op0=ALU.is_equal)
            if nsh > nn:
                tg_ps2 = psum_s.tile([T, ncol], f32, tag="gates")
                nc.tensor.matmul(tg_ps2[:, 0:nsh - nn], ones1_20[:], tgc[:, nn:nsh],
                                 start=True, stop=True)
                nc.vector.tensor_scalar(M1c[:, nn:nsh], tg_ps2[:, 0:nsh - nn],
                                        iota20[:], None, op0=ALU.is_equal)
            # emission part of score
            EMSc = work.tile([T, ncol], f32, tag="EMSc")
            nc.vector.tensor_mul(EMSc[:, 0:nn], em_ps[:, 0:nn], M1c[:, 0:nn])
            nc.vector.reduce_sum(red[:],
                                 EMSc[:, 0:nn].rearrange("p (t b) -> p b t", b=BC),
                                 axis=mybir.AxisListType.X)
            nc.vector.tensor_add(pile[:], pile[:], red[:])
            # transition part: G[:,tb]*M1[:,tb+BC], tb in [n0, n0+nn-?)
            g_ps = psum_s.tile([T, ncol], f32, tag="gates")
            nc.tensor.matmul(g_ps[:, 0:nn], transb[:], M1c[:, 0:nn],
                             start=True, stop=True)
            n3 = nn if n0 + nn < ntb else nn - BC
            if n3 > 0:
                S3c = work.tile([T, ncol], f32, tag="EMSc")
                nc.vector.tensor_mul(S3c[:, 0:n3], g_ps[:, 0:n3], M1c[:, BC:BC + n3])
                nc.vector.reduce_sum(red[:],
                                     S3c[:, 0:n3].rearrange("p (t b) -> p b t", b=BC),
                                     axis=mybir.AxisListType.X)
                nc.vector.tensor_add(pile[:], pile[:], red[:])
            # start / end parts
            if ci == 0:
                nc.vector.tensor_scalar(red[:], M1c[:, 0:BC], startT[:], None,
                                        op0=ALU.mult)
                nc.vector.tensor_add(pile[:], pile[:], red[:])
            if n0 + nn == ntb:
                nc.vector.tensor_scalar(red[:], M1c[:, nn - BC:nn], endT[:], None,
                                        op0=ALU.mult)
                nc.vector.tensor_add(pile[:], pile[:], red[:])
        sc_ps = psum_s.tile([1, BC], f32, tag="small")
        nc.tensor.matmul(sc_ps[:], ones20f[:], pile[:], start=True, stop=True)
        score = state.tile([1, BC], f32, tag="score")
        nc.vector.tensor_copy(score[:], sc_ps[:])
        if "score" in dbg:
            nc.sync.dma_start(dbg["score"][:], score[:])

        # ---------- P5: exp-space forward scan ----------
        STEPS_PER_ECH = ncol // BC        # 32 scan steps per expem chunk

        def loadch(k):
            t_ = stage.tile([T, ncol], bf16, tag="exc")
            nc.sync.dma_start(t_[:], expem_d[:, k * ncol:(k + 1) * ncol])
            return t_

        ex_cur = loadch(0)
        ex_nxt = loadch(1) if nchunks > 1 else None
        Scur = state.tile([T, BC], f32, tag="S0")
        nc.vector.tensor_scalar(Scur[:], ex_cur[:, 0:BC], startexp[:], None,
                                op0=ALU.mult)
        for t in range(1, Lsteps):
            k, off = divmod(t, STEPS_PER_ECH)
            if off == 0:
                ex_cur = ex_nxt
                if k + 1 < nchunks:
                    ex_nxt = loadch(k + 1)
            sp = psum_s.tile([T, BC], f32, tag="small")
            nc.tensor.matmul(sp[:], mexp[:], Scur[:], start=True, stop=True)
            Snew = work.tile([T, BC], f32, tag="Sflip")
            nc.vector.tensor_mul(Snew[:], sp[:], ex_cur[:, off * BC:(off + 1) * BC])
            Scur = Snew
        if "score" in dbg:
            nc.sync.dma_start(dbg["SL"][:], Scur[:])
        EE = state.tile([T, BC], f32, tag="EE")
        nc.vector.tensor_scalar(EE[:], Scur[:], endexp[:], None, op0=ALU.mult)
        z_ps = psum_s.tile([1, BC], f32, tag="small")
        nc.tensor.matmul(z_ps[:], ones20f[:], EE[:], start=True, stop=True)
        vt = state.tile([1, BC], f32, tag="vt")
        nc.scalar.activation(vt[:], z_ps[:], AF.Ln)
        nc.vector.tensor_sub(vt[:], vt[:], score[:])
        nc.sync.dma_start(v_o[:], vt[:])
    nc.compile()
    return nc


def _host_inputs_from_dict(np_in, Lsteps):
    f = lambda k: np.asarray(np_in[k], np.float32)
    return _build_host_inputs(
        np.asarray(np_in["x"]), np.asarray(np_in["tags"]), f("emb"),
        f("w_ih0"), f("w_hh0"), f("b_ih0"), f("b_hh0"),
        f("w_ih1"), f("w_hh1"), f("b_ih1"), f("b_hh1"),
        f("fc_W"), f("fc_b"), f("crf_trans"), f("crf_start"), f("crf_end"),
        Lsteps)


TRACE = False          # set by test harnesses to capture an NTFF profile
LAST_RESULTS = None


def kernel(**inputs):
    global LAST_RESULTS
    np_in = {k: np.asarray(v) for k, v in inputs.items()}
    shared, per_core = _host_inputs_from_dict(np_in, L)
    nc = build_nc(L)
    in_maps = [dict(shared, **pc) for pc in per_core]
    LAST_RESULTS = run_bass_kernel_spmd(nc, in_maps, list(range(NC_CORES)),
                                        trace=TRACE)
    v = np.concatenate([r["v"][0] for r in LAST_RESULTS.results])
    return np.float32(np.mean(v))

